# revision 29
# baseline (speedup 1.0000x reference)
"""V14: batched front-end (one v/h2 add over all 4 images, xor/exp/ln in
2-image pairs), per-image matmuls + fused boundary evals, 3-ring DMA with
host partition-major prepack, exp+ln pinned to one ACT table.

Math: bce = softplus((1-2t)*x) = ln(1 + exp(x XOR mask)), mask = t ? 0x8000 : 0
(host-encoded).  Box sum s of t over 5x5 via one horizontal pair-add
v = t0+t2, h2 = v+t4 (DVE) and a banded matmul pair per window
(s = band^T @ h2 + band^T @ v shifted); weight w = 5 - 4*nb with
nb = (s==0 | s==25).  Per image, two fused DVE passes accumulate
E = sum((s==25)*spy) and Z = sum((s==0)*spy) with exact 0/1 products;
sum(w*spy) = 5*S - 4*E - 4*Z.

Layout per core (4 images): 4 main windows of 124 out rows at 124-row
stride; t window tiles carry rows 124w-2..124w+125 at partitions 0..127,
pre-padded on host; x / mask / spy / psum-out at partitions 0..123.
16-row tail batched across the 4 images.
"""

import os

os.environ.setdefault("TILE_SCHEDULER", "asap")

import numpy as np

import concourse.bass as bass
import concourse.bacc as bacc_mod
import concourse.tile as tile
import concourse.hw_specs as hw_specs
from concourse import mybir
from concourse.bass_utils import run_bass_kernel_spmd

F32 = mybir.dt.float32
BF16 = mybir.dt.bfloat16
U16 = mybir.dt.uint16
ALU = mybir.AluOpType
ACTF = mybir.ActivationFunctionType

B, H, W = 32, 512, 512
HW = H * W
NCORES = 8
IMGS = B // NCORES
NW = 4                      # main windows, 124 out rows each
TW = 516                    # padded width
TAIL_P = 22                 # tail t partitions (t rows 494..515)
NSLOT = IMGS + 1

_orig_get_tables = hw_specs.get_activation_tables


def _tables_pin_ln_exp(arch):
    # Keep original set order (walrus resolves set ids by position) but make
    # natural_log_exp_and_others the only provider of Exp/Ln: one table, no
    # reloads.
    tabs = _orig_get_tables(arch)
    keep = "natural_log_exp_and_others"
    out = {}
    for k, v in tabs.items():
        if k != keep:
            v = v - {ACTF.Exp, ACTF.Ln}
        out[k] = v
    return out


bacc_mod.get_activation_tables = _tables_pin_ln_exp


def _make_band() -> np.ndarray:
    band = np.zeros((128, 124), dtype=np.float32)
    for m in range(124):
        band[m: m + 5, m] = 1.0
    return band


def _ap(t, off, dims):
    return bass.AP(t, off, dims)


def _build_nc() -> bass.Bass:
    nc = bacc_mod.Bacc(trn_type="TRN2")

    twin = nc.dram_tensor("twin", [IMGS, 128, NW, TW], BF16, kind="ExternalInput")
    xw = nc.dram_tensor("xw", [IMGS, 124, NW, W], BF16, kind="ExternalInput")
    mkw = nc.dram_tensor("mkw", [IMGS, 124, NW, W], U16, kind="ExternalInput")
    ttl = nc.dram_tensor("ttl", [TAIL_P, IMGS, TW], BF16, kind="ExternalInput")
    xtl = nc.dram_tensor("xtl", [16, IMGS, W], BF16, kind="ExternalInput")
    mktl = nc.dram_tensor("mktl", [16, IMGS, W], U16, kind="ExternalInput")
    band = nc.dram_tensor("band", [128, 124], BF16, kind="ExternalInput")
    out_sp = nc.dram_tensor("out_sp", [128, NSLOT], F32, kind="ExternalOutput")
    out_r = nc.dram_tensor("out_r", [128, NSLOT], F32, kind="ExternalOutput")
    out_n = nc.dram_tensor("out_n", [128, NSLOT], F32, kind="ExternalOutput")

    with tile.TileContext(nc) as tc:
        with (
            tc.tile_pool(name="singles", bufs=1) as singles,
            tc.tile_pool(name="scrp", bufs=4) as scrp,
            tc.tile_pool(name="psum", bufs=2, space="PSUM") as psum,
        ):
            band_sb = singles.tile([128, 124], BF16)
            nc.scalar.dma_start(band_sb[:], band[:])

            stats_sp = singles.tile([128, NSLOT], F32)
            stats_r = singles.tile([128, NSLOT], F32)
            stats_n = singles.tile([128, NSLOT], F32)
            nc.vector.memset(stats_sp[:], 0.0)
            nc.vector.memset(stats_r[:], 0.0)
            nc.vector.memset(stats_n[:], 0.0)

            # batched input tiles
            t_sb = singles.tile([128, IMGS, NW, TW], BF16)
            x_sb = singles.tile([124, IMGS, NW, W], BF16)
            mk_sb = singles.tile([124, IMGS, NW, W], U16)
            t_tl = singles.tile([TAIL_P, IMGS, TW], BF16)
            x_tl = singles.tile([16, IMGS, W], BF16)
            mk_tl = singles.tile([16, IMGS, W], U16)

            for i in range(IMGS):
                nc.sync.dma_start(
                    t_sb[:, i, :, :],
                    _ap(twin, i * 128 * NW * TW, [[NW * TW, 128], [TW, NW], [1, TW]]),
                )
                nc.scalar.dma_start(
                    x_sb[:, i, :, :],
                    _ap(xw, i * 124 * NW * W, [[NW * W, 124], [W, NW], [1, W]]),
                )
                nc.gpsimd.dma_start(
                    mk_sb[:, i, :, :],
                    _ap(mkw, i * 124 * NW * W, [[NW * W, 124], [W, NW], [1, W]]),
                )
            nc.sync.dma_start(
                t_tl[:], _ap(ttl, 0, [[IMGS * TW, TAIL_P], [TW, IMGS], [1, TW]])
            )
            nc.scalar.dma_start(
                x_tl[:], _ap(xtl, 0, [[IMGS * W, 16], [W, IMGS], [1, W]])
            )
            nc.gpsimd.dma_start(
                mk_tl[:], _ap(mktl, 0, [[IMGS * W, 16], [W, IMGS], [1, W]])
            )

            # ---- batched front-end
            v_all = singles.tile([128, IMGS, NW, W + 2], BF16)
            nc.vector.tensor_tensor(
                v_all[:], t_sb[:, :, :, 0:514], t_sb[:, :, :, 2:516], op=ALU.add
            )
            h_all = singles.tile([128, IMGS, NW, W], BF16)
            nc.vector.tensor_tensor(
                h_all[:], v_all[:, :, :, 0:512], t_sb[:, :, :, 4:516], op=ALU.add
            )
            y_all = singles.tile([124, IMGS, NW, W], BF16)
            ey_all = singles.tile([124, IMGS, NW, W], BF16)
            spy_all = singles.tile([124, IMGS, NW, W], BF16)
            for p in range(2):
                pi = slice(2 * p, 2 * p + 2)
                nc.vector.tensor_tensor(
                    y_all[:, pi, :, :].bitcast(U16),
                    mk_sb[:, pi, :, :],
                    x_sb[:, pi, :, :].bitcast(U16),
                    op=ALU.bitwise_xor,
                )
                nc.scalar.activation(
                    ey_all[:, pi, :, :], y_all[:, pi, :, :], ACTF.Exp
                )
                nc.scalar.activation(
                    spy_all[:, pi, :, :],
                    ey_all[:, pi, :, :],
                    ACTF.Ln,
                    bias=1.0,
                    accum_out=stats_sp[0:124, p: p + 1],
                )

            # tail front-end
            v_tl = singles.tile([TAIL_P, IMGS, W + 2], BF16)
            nc.vector.tensor_tensor(
                v_tl[:], t_tl[:, :, 0:514], t_tl[:, :, 2:516], op=ALU.add
            )
            h_tl = singles.tile([TAIL_P, IMGS, W], BF16)
            nc.vector.tensor_tensor(
                h_tl[:], v_tl[:, :, 0:512], t_tl[:, :, 4:516], op=ALU.add
            )
            y_tl = singles.tile([16, IMGS, W], BF16)
            nc.vector.tensor_tensor(
                y_tl[:].bitcast(U16),
                mk_tl[:],
                x_tl[:].bitcast(U16),
                op=ALU.bitwise_xor,
            )
            ey_tl = singles.tile([16, IMGS, W], BF16)
            nc.scalar.activation(ey_tl[:], y_tl[:], ACTF.Exp)
            spy_tl = singles.tile([16, IMGS, W], BF16)
            nc.scalar.activation(
                spy_tl[:],
                ey_tl[:],
                ACTF.Ln,
                bias=1.0,
                accum_out=stats_sp[0:16, 2:3],
            )

            # ---- per-image matmuls + fused boundary accumulation
            for i in range(IMGS):
                s_ps = psum.tile([124, NW, W], F32, tag="s")
                for w in range(NW):
                    nc.tensor.matmul(
                        s_ps[:, w, :], band_sb[:], h_all[:, i, w, :],
                        start=True, stop=False,
                    )
                    nc.tensor.matmul(
                        s_ps[:, w, :], band_sb[:], v_all[:, i, w, 1:513],
                        start=False, stop=True,
                    )
                scrv = scrp.tile([124, NW, W], BF16, tag="scr")
                nc.vector.scalar_tensor_tensor(
                    scrv[:],
                    s_ps[:],
                    25.0,
                    spy_all[:, i, :, :],
                    op0=ALU.is_equal,
                    op1=ALU.mult,
                    accum_out=stats_r[0:124, i: i + 1],
                )
                scrn = scrp.tile([124, NW, W], BF16, tag="scr")
                nc.vector.scalar_tensor_tensor(
                    scrn[:],
                    s_ps[:],
                    0.0,
                    spy_all[:, i, :, :],
                    op0=ALU.is_equal,
                    op1=ALU.mult,
                    accum_out=stats_n[0:124, i: i + 1],
                )

            s_tl = psum.tile([16, IMGS, W], F32, tag="s")
            for i in range(IMGS):
                nc.tensor.matmul(
                    s_tl[:, i, :], band_sb[0:TAIL_P, 0:16], h_tl[:, i, :],
                    start=True, stop=False,
                )
                nc.tensor.matmul(
                    s_tl[:, i, :], band_sb[0:TAIL_P, 0:16], v_tl[:, i, 1:513],
                    start=False, stop=True,
                )
            scr_tl = scrp.tile([16, IMGS, W], BF16, tag="scr")
            nc.vector.scalar_tensor_tensor(
                scr_tl[:],
                s_tl[:],
                25.0,
                spy_tl[:],
                op0=ALU.is_equal,
                op1=ALU.mult,
                accum_out=stats_r[0:16, IMGS: IMGS + 1],
            )
            scrn_tl = scrp.tile([16, IMGS, W], BF16, tag="scr")
            nc.vector.scalar_tensor_tensor(
                scrn_tl[:],
                s_tl[:],
                0.0,
                spy_tl[:],
                op0=ALU.is_equal,
                op1=ALU.mult,
                accum_out=stats_n[0:16, IMGS: IMGS + 1],
            )

            nc.sync.dma_start(out_sp[:], stats_sp[:])
            nc.sync.dma_start(out_r[:], stats_r[:])
            nc.sync.dma_start(out_n[:], stats_n[:])

    nc.compile()
    nc.finalize()
    return nc


_NC = None


def _get_nc() -> bass.Bass:
    global _NC
    if _NC is None:
        _NC = _build_nc()
    return _NC


def _make_in_maps(pred: np.ndarray, target: np.ndarray) -> list[dict]:
    import ml_dtypes

    bf16 = ml_dtypes.bfloat16
    pred = np.ascontiguousarray(pred.reshape(B, H, W)).astype(bf16)
    target = target.reshape(B, H, W)
    tb = target.astype(bf16)
    mask_full = np.where(target > 0.5, np.uint16(0x8000), np.uint16(0)).astype(
        np.uint16
    )
    band = _make_band().astype(bf16)

    twin = np.zeros((B, 128, NW, TW), dtype=bf16)
    for w in range(NW):
        r0 = 124 * w - 2
        pl = max(0, -r0)
        twin[:, pl:128, w, 2:514] = tb[:, r0 + pl: r0 + 128, :]
    perm = (
        lambda a: np.ascontiguousarray(
            a[:, 0: 4 * 124, :].reshape(B, NW, 124, W).transpose(0, 2, 1, 3)
        )
    )
    xw = perm(pred)
    mkw = perm(mask_full)
    ttl = np.zeros((TAIL_P, B, TW), dtype=bf16)
    ttl[0:18, :, 2:514] = tb[:, 494:512, :].transpose(1, 0, 2)
    xtl = np.ascontiguousarray(pred[:, 496:512, :].transpose(1, 0, 2))
    mktl = np.ascontiguousarray(mask_full[:, 496:512, :].transpose(1, 0, 2))

    in_maps = []
    for c in range(NCORES):
        sl = slice(c * IMGS, (c + 1) * IMGS)
        in_maps.append(
            {
                "twin": np.ascontiguousarray(twin[sl]),
                "xw": xw[sl],
                "mkw": mkw[sl],
                "ttl": np.ascontiguousarray(ttl[:, sl]),
                "xtl": np.ascontiguousarray(xtl[:, sl]),
                "mktl": np.ascontiguousarray(mktl[:, sl]),
                "band": band,
            }
        )
    return in_maps


def _finish(results: list[dict]) -> np.ndarray:
    total = 0.0
    for res in results:
        total += 5.0 * np.sum(res["out_sp"], dtype=np.float64)
        total -= 4.0 * np.sum(res["out_r"], dtype=np.float64)
        total -= 4.0 * np.sum(res["out_n"], dtype=np.float64)
    mean = total / float(B * H * W)
    return np.asarray(np.float32(mean))


def kernel(pred: np.ndarray, target: np.ndarray, **run_kwargs) -> np.ndarray:
    pred = np.asarray(pred)
    target = np.asarray(target)
    nc = _get_nc()
    in_maps = _make_in_maps(pred, target)
    out = run_bass_kernel_spmd(nc, in_maps, core_ids=list(range(NCORES)), **run_kwargs)
    res = _finish(out.results)
    kernel.last_run = out
    return res
